# revision 1
# baseline (speedup 1.0000x reference)
"""Trainium2 Bass kernel for nn_DWTFeatureModel.

Pipeline: x (N,1,512,8,8) -> maxpool(1,2,2) -> per-128-sample-subwindow DWT(db4, J=4)
-> per-bin full-kernel Conv3d -> bias -> LeakyReLU(0.02) -> (N, 192).

Key algebraic fold: everything after the maxpool is linear in the pooled
signal, so DWT+conv collapse into one matmul with precombined weights
  Weff[b, s, hw, f] = sum_t DWTmat[s, t] * conv_w[b, f, t, h, w].

Sharding: pure data parallelism, batch 2048 -> 8 cores x 256.

Per-core dataflow:
  DMA x in tiles (128 batch partitions, 64 timesteps x 64 spatial free)
  -> VectorE 2x strided tensor_max = maxpool 2x2
  -> TensorE transpose (128x128 blocks) to put (time,space) on partitions
  -> ScalarE copy PSUM->SBUF
  -> TensorE accumulating matmuls vs Weff (+ ones-row matmul for bias)
  -> ScalarE LeakyReLU -> DMA out.
"""

import numpy as np

N_CORES = 8
N_FULL = 2048
N_PER = N_FULL // N_CORES          # 256
NB = N_PER // 128                  # 2 n-blocks per core
NCH = 8                            # time chunks of 64 (512 total)
TC = 64                            # timesteps per chunk
NF = 48
NBINS = 4
OUTF = NBINS * NF                  # 192
NEG = 0.02

# ---- db4 analysis filters (pywt), reversed for cross-correlation ----
_DEC_LO = np.array([-0.010597401784997278, 0.032883011666982945,
                    0.030841381835986965, -0.18703481171888114,
                    -0.02798376941698385, 0.6308807679295904,
                    0.7148465705525415, 0.23037781330885523], np.float64)
_DEC_HI = np.array([-0.23037781330885523, 0.7148465705525415,
                    -0.6308807679295904, -0.02798376941698385,
                    0.18703481171888114, 0.030841381835986965,
                    -0.032883011666982945, -0.010597401784997278], np.float64)
_H0R = _DEC_LO[::-1].copy()
_H1R = _DEC_HI[::-1].copy()
_L = 8
_J = 4


def _afb1d_np(x):
    N = x.shape[-1]
    out = (N + _L - 1) // 2
    p = 2 * (out - 1) - N + _L
    xp = np.pad(x, ((0, 0), (p // 2, (p + 1) // 2)), mode="reflect")
    lo = np.empty((x.shape[0], out), np.float64)
    hi = np.empty((x.shape[0], out), np.float64)
    for i in range(out):
        seg = xp[:, 2 * i:2 * i + _L]
        lo[:, i] = seg @ _H0R
        hi[:, i] = seg @ _H1R
    return lo, hi


def _dwt_matrix():
    """(128, 154): row s = DWT coefficients of the unit impulse at position s."""
    his = []
    lo = np.eye(128)
    for _ in range(_J):
        lo, hi = _afb1d_np(lo)
        his.append(hi)
    return np.concatenate([lo] + his, axis=-1)


_DWT_M = _dwt_matrix()


def _prepare_weights(conv_w, conv_b):
    """Fold DWT into conv weights, layout for the on-chip matmul.

    Returns
      wall: (128, 64*48) f32.  Partition m = j*16 + hw  (j = t mod 8).
            Free block cg = ch*8+ct covers timesteps t = cg*8 .. cg*8+7,
            i.e. bin b = cg//16, s = (cg%16)*8 + j.
      bias: (1, 192) f32, bin-major.
    """
    M = _DWT_M.astype(np.float64)
    cw = conv_w.astype(np.float64)                       # (4, 48, 154, 4, 4)
    weff = np.einsum("st,bfthw->bshwf", M, cw)           # (4, 128, 4, 4, 48)
    weff = weff.reshape(4, 2, 8, 8, 16, 48)              # b, q2, ct, j, hw, f
    wall = weff.transpose(3, 4, 0, 1, 2, 5).reshape(128, 64 * 48)
    return np.ascontiguousarray(wall, np.float32), \
        np.ascontiguousarray(conv_b.reshape(1, OUTF), np.float32)


_NC_CACHE = {}

# tuning knobs (HW A/B'd via R-loop slope benchmark, 2026-08-04)
RAW_BUFS = 6
# 7x64t chunks then 48+16: the final small chunk shrinks the end-of-stream
# compute tail (pool+transpose+matmul chain) that runs after the last DMA
CHUNK_SCHED = [64, 64, 64, 64, 64, 64, 64, 48, 16]
POOL_UNIT = 64           # timesteps per maxpool/compute unit within a chunk
ALT_RINGS = False        # alternate input DMAs between the two HWDGE rings
ALT_GPSIMD = False       # alternate input DMAs between SP-HWDGE and GpSimd-SWDGE
LAST_POOL_GPSIMD = False # run the final chunk's maxpool on idle GpSimd engine


def _build_bass(loop_r=None, consts_in_loop=False, const_eng="sp"):
    import concourse.bass as bass
    import concourse.bacc as bacc
    import concourse.mybir as mybir
    import concourse.tile as tile

    f32 = mybir.dt.float32
    nc = bacc.Bacc()

    x_d = nc.dram_tensor("x", [N_PER, 1, 512, 8, 8], f32, kind="ExternalInput")
    w_d = nc.dram_tensor("wall", [128, 64 * NF], f32, kind="ExternalInput")
    bias_d = nc.dram_tensor("bias", [1, OUTF], f32, kind="ExternalInput")
    ident_d = nc.dram_tensor("ident", [128, 128], f32, kind="ExternalInput")
    ones_d = nc.dram_tensor("ones", [1, 128], f32, kind="ExternalInput")
    out_d = nc.dram_tensor("out", [N_PER, OUTF], f32, kind="ExternalOutput")

    assert sum(CHUNK_SCHED) == 512 and all(c % 8 == 0 for c in CHUNK_SCHED)

    # HBM view: (n, t, h*w); per-(n, chunk) runs are tc*256B contiguous
    x_flat = x_d.rearrange("n one t h w -> n t (one h w)")

    import contextlib
    sizes = sorted(set(CHUNK_SCHED))
    with tile.TileContext(nc) as tc, contextlib.ExitStack() as ctx:
        consts = ctx.enter_context(tc.tile_pool(name="consts", bufs=1))
        def _raw_bufs(s):
            if len(sizes) == 1:
                return RAW_BUFS
            return 3 if s == max(sizes) else 2
        rawps = {
            s: ctx.enter_context(tc.tile_pool(name=f"raw{s}", bufs=_raw_bufs(s)))
            for s in sizes
        }
        o1p = ctx.enter_context(tc.tile_pool(name="o1", bufs=2))
        pooledp = ctx.enter_context(tc.tile_pool(name="pooled", bufs=2))
        tsbp = ctx.enter_context(tc.tile_pool(name="tsb", bufs=6))
        outp = ctx.enter_context(tc.tile_pool(name="outp", bufs=2))
        tpp = ctx.enter_context(tc.tile_pool(name="tp", bufs=4,
                                             space=bass.MemorySpace.PSUM))
        accp = ctx.enter_context(tc.tile_pool(name="acc", bufs=2,
                                              space=bass.MemorySpace.PSUM))
        if True:
            # Pre-issue the first input chunk's DMA so the 1.6MB constants
            # upload doesn't delay the (critical-path) input stream. The
            # constants are not needed until the first matmul ~10us in.
            tcl0 = CHUNK_SCHED[0]
            raw0 = rawps[tcl0].tile([128, tcl0 * 64], f32, tag="raw")
            src0 = x_flat[0:128, 0:tcl0, :]
            nc.sync.dma_start(raw0[:], src0.rearrange("p t e -> p (t e)"))

            # constants stay on the SP HWDGE ring with the input stream:
            # measured on HW, the ACT ring path costs ~2us per DMA extra
            # (see memory: alternate-ring DMA consistently regresses here)
            w_t = consts.tile([128, 64 * NF], f32)
            bias_t = consts.tile([1, OUTF], f32)
            ident_t = consts.tile([128, 128], f32)
            ones_t = consts.tile([1, 128], f32)

            def emit_consts():
                ceng = nc.scalar if const_eng == "act" else nc.sync
                ceng.dma_start(w_t[:], w_d[:])
                ceng.dma_start(bias_t[:], bias_d[:])
                ceng.dma_start(ident_t[:], ident_d[:])
                ceng.dma_start(ones_t[:], ones_d[:])

            if not consts_in_loop:
                emit_consts()

            loop_cm = tc.For_i(0, loop_r, 1) if loop_r else contextlib.nullcontext()
            with loop_cm:
                if consts_in_loop:
                    emit_consts()
                _kernel_body(nc, tc, mybir, f32, x_flat, w_t, bias_t, ident_t,
                             ones_t, out_d, rawps, o1p, pooledp, tsbp, outp,
                             tpp, accp, raw0=None if loop_r else raw0)

    nc.compile()
    return nc


def _kernel_body(nc, tc, mybir, f32, x_flat, w_t, bias_t, ident_t, ones_t,
                 out_d, rawps, o1p, pooledp, tsbp, outp, tpp, accp, raw0=None):
    for nb in range(NB):
        acc = accp.tile([128, OUTF], f32)
        t0 = 0
        for ch, tcl in enumerate(CHUNK_SCHED):
            if nb == 0 and ch == 0 and raw0 is not None:
                raw = raw0       # DMA already issued before the consts load
            else:
                raw = rawps[tcl].tile([128, tcl * 64], f32, tag="raw")
                if ALT_GPSIMD and ch % 2:
                    eng = nc.gpsimd
                elif ALT_RINGS and ch % 2:
                    eng = nc.scalar
                else:
                    eng = nc.sync
                src_ap = x_flat[nb * 128:(nb + 1) * 128, t0:t0 + tcl, :]
                eng.dma_start(raw[:], src_ap.rearrange("p t e -> p (t e)"))

            last_chunk = (nb == NB - 1 and ch == len(CHUNK_SCHED) - 1)
            pool_eng = (nc.gpsimd if (LAST_POOL_GPSIMD and last_chunk)
                        else nc.vector)
            pu = min(POOL_UNIT, tcl)
            for u in range(tcl // pu):
                # maxpool over w-pairs (adjacent elements)
                o1 = o1p.tile([128, pu * 32], f32, tag=f"o1{pu}")
                r2 = raw[:, u * pu * 64:(u + 1) * pu * 64].rearrange(
                    "p (m two) -> p m two", two=2)
                pool_eng.tensor_max(o1[:], r2[:, :, 0], r2[:, :, 1])

                # maxpool over h-pairs: o1 layout (q, h, ww) -> (blk, hp, ww)
                pooled = pooledp.tile([128, pu * 16], f32, tag=f"pl{pu}")
                o3 = o1.rearrange("p (blk hp ww) -> p blk hp ww", hp=2, ww=4)
                pool_eng.tensor_max(pooled[:], o3[:, :, 0, :], o3[:, :, 1, :])

                for ct in range(pu // 8):
                    cg = (t0 + u * pu) // 8 + ct   # global 8-t block, 0..63
                    b = cg // 16
                    if cg % 16 == 0:
                        # open this bin's accumulation group with the bias row
                        nc.tensor.matmul(
                            acc[:, NF * b:NF * (b + 1)], ones_t[:],
                            bias_t[:, NF * b:NF * (b + 1)],
                            start=True, stop=False)
                    tp = tpp.tile([128, 128], f32)
                    nc.tensor.transpose(tp[:], pooled[:, ct * 128:(ct + 1) * 128],
                                        ident_t[:])
                    ts = tsbp.tile([128, 128], f32)
                    nc.scalar.copy(ts[:], tp[:])
                    nc.tensor.matmul(
                        acc[:, NF * b:NF * (b + 1)], ts[:],
                        w_t[:, NF * cg:NF * (cg + 1)],
                        start=False, stop=(cg % 16 == 15))
            t0 += tcl

        # LeakyReLU(z) = max(0.02*z, z) for slope < 1. Both ops on DVE:
        # same-engine in-order execution avoids a cross-engine sem hop in
        # the end-of-kernel critical tail.
        sc = outp.tile([128, OUTF], f32, tag="sc")
        nc.vector.tensor_scalar_mul(sc[:], acc[:], NEG)   # PSUM -> SBUF, *0.02
        ot = outp.tile([128, OUTF], f32, tag="ot")
        nc.vector.tensor_max(ot[:], acc[:], sc[:])
        nc.sync.dma_start(out_d[nb * 128:(nb + 1) * 128, :], ot[:])


def _import_concourse():
    try:
        import concourse.bass_utils  # noqa: F401
    except ImportError:
        import sys
        for p in ("/opt/trn_rl_repo", "/root/.axon_site/_ro/trn_rl_repo"):
            if p not in sys.path:
                sys.path.insert(0, p)
        import concourse.bass_utils  # noqa: F401


def kernel(x, conv_w, conv_b):
    _import_concourse()
    from concourse.bass_utils import run_bass_kernel_spmd

    x = np.ascontiguousarray(np.asarray(x), np.float32)
    wall, bias = _prepare_weights(np.asarray(conv_w), np.asarray(conv_b))
    ident = np.eye(128, dtype=np.float32)
    ones = np.ones((1, 128), np.float32)

    if "nc" not in _NC_CACHE:
        _NC_CACHE["nc"] = _build_bass()
    nc = _NC_CACHE["nc"]

    in_maps = [
        {"x": np.ascontiguousarray(x[i * N_PER:(i + 1) * N_PER]),
         "wall": wall, "bias": bias, "ident": ident, "ones": ones}
        for i in range(N_CORES)
    ]
    res = run_bass_kernel_spmd(nc, in_maps, list(range(N_CORES)))
    return np.concatenate([res.results[i]["out"] for i in range(N_CORES)], axis=0)



# revision 35
# speedup vs baseline: 1.0427x; 1.0427x over previous
"""Trainium2 Bass kernel for nn_DWTFeatureModel.

Pipeline: x (N,1,512,8,8) -> maxpool(1,2,2) -> per-128-sample-subwindow DWT(db4, J=4)
-> per-bin full-kernel Conv3d -> bias -> LeakyReLU(0.02) -> (N, 192).

Key algebraic fold: everything after the maxpool is linear in the pooled
signal, so DWT+conv collapse into one matmul with precombined weights
  Weff[b, s, hw, f] = sum_t DWTmat[s, t] * conv_w[b, f, t, h, w].

Sharding: pure data parallelism, batch 2048 -> 8 cores x 256.

Per-core dataflow: two sequential 128-partition n-blocks, input streamed
on the SP HWDGE queue in 64-timestep chunks (16KB/partition descriptors —
measured best; the stream runs at ~334 GB/s, the practical HBM limit here).
Constants load on the idle GpSimd SWDGE queue so they never sit ahead of
input chunks on the critical queue. The matmul path runs in bf16
(pooled data + folded weights; rel_err ~2e-3 vs the 2e-2 budget), which
halves on-chip SBUF traffic and measures ~2us faster than f32.

  DMA x chunks (128 batch partitions, tcl timesteps x 64 spatial)
  -> VectorE 2x strided tensor_max = maxpool 2x2 (bf16 out)
  -> TensorE transpose (128x128 blocks) to put (time,space) on partitions
  -> ScalarE copy PSUM->SBUF
  -> TensorE accumulating bf16 matmuls vs Weff (+ ones-row matmul for bias)
  -> VectorE LeakyReLU -> DMA out.

Measured dead ends (see memory): dual-queue input streaming (+15-19us with
compute despite being neutral DMA-only), 128t chunks, gpsimd compute ops
(compile failure), DVE-hosted PSUM copies, per-bin output drain.
"""

import numpy as np

N_CORES = 8
N_FULL = 2048
N_PER = N_FULL // N_CORES          # 256
NB = N_PER // 128                  # 2 n-blocks per core
NF = 48
NBINS = 4
OUTF = NBINS * NF                  # 192
NEG = 0.02

# ---- db4 analysis filters (pywt), reversed for cross-correlation ----
_DEC_LO = np.array([-0.010597401784997278, 0.032883011666982945,
                    0.030841381835986965, -0.18703481171888114,
                    -0.02798376941698385, 0.6308807679295904,
                    0.7148465705525415, 0.23037781330885523], np.float64)
_DEC_HI = np.array([-0.23037781330885523, 0.7148465705525415,
                    -0.6308807679295904, -0.02798376941698385,
                    0.18703481171888114, 0.030841381835986965,
                    -0.032883011666982945, -0.010597401784997278], np.float64)
_H0R = _DEC_LO[::-1].copy()
_H1R = _DEC_HI[::-1].copy()
_L = 8
_J = 4


def _afb1d_np(x):
    N = x.shape[-1]
    out = (N + _L - 1) // 2
    p = 2 * (out - 1) - N + _L
    xp = np.pad(x, ((0, 0), (p // 2, (p + 1) // 2)), mode="reflect")
    lo = np.empty((x.shape[0], out), np.float64)
    hi = np.empty((x.shape[0], out), np.float64)
    for i in range(out):
        seg = xp[:, 2 * i:2 * i + _L]
        lo[:, i] = seg @ _H0R
        hi[:, i] = seg @ _H1R
    return lo, hi


def _dwt_matrix():
    """(128, 154): row s = DWT coefficients of the unit impulse at position s."""
    his = []
    lo = np.eye(128)
    for _ in range(_J):
        lo, hi = _afb1d_np(lo)
        his.append(hi)
    return np.concatenate([lo] + his, axis=-1)


_DWT_M = _dwt_matrix()


def _prepare_weights(conv_w, conv_b):
    """Fold DWT into conv weights, layout for the on-chip matmul.

    Returns
      wall: (128, 64*48) f32.  Partition m = j*16 + hw  (j = t mod 8).
            Free block cg = ch*8+ct covers timesteps t = cg*8 .. cg*8+7,
            i.e. bin b = cg//16, s = (cg%16)*8 + j.
      bias: (1, 192) f32, bin-major.
    """
    M = _DWT_M.astype(np.float64)
    cw = conv_w.astype(np.float64)                       # (4, 48, 154, 4, 4)
    weff = np.einsum("st,bfthw->bshwf", M, cw)           # (4, 128, 4, 4, 48)
    weff = weff.reshape(4, 2, 8, 8, 16, 48)              # b, q2, ct, j, hw, f
    wall = weff.transpose(3, 4, 0, 1, 2, 5).reshape(128, 64 * 48)
    return np.ascontiguousarray(wall, np.float32), \
        np.ascontiguousarray(conv_b.reshape(1, OUTF), np.float32)


_NC_CACHE = {}

# ---- tuning knobs (HW A/B'd via R-loop slope benchmark, 2026-08-08/09) ----
STREAM_ENGS = ("sync", "sync")  # all input DMA on the SP queue (dual regresses)
OUT_ENGS = ("sync", "sync")
CONST_ENG = "gpsimd"         # constants off the input queue entirely (SWDGE)
CHUNK_SCHED = [64, 64, 64, 64, 64, 64, 64, 48, 16]
POOL_UNIT = 64               # timesteps per maxpool/compute unit within a chunk
RAW_BUFS_BIG = 3             # per-stream bufs for the largest chunk size
RAW_BUFS_SMALL = 2           # shared-across-streams bufs for tail sizes
INTERLEAVE = False           # process the two n-blocks sequentially
DMA_ONLY = False             # microbench: input stream only, no compute
BF16_MM = True               # bf16 pooled/weights for the transpose+matmul path
SHARE_BIG = False            # share the big-chunk raw pool across streams
COPY_ENG = "scalar"          # engine for the PSUM->SBUF transpose copies
POOL2_ENG = "vector"         # engine for the second maxpool stage
O1_BF16 = True               # bf16 for the first maxpool stage output
OUT_PER_BIN = False          # drain each bin's output as its accum group closes


def _build_bass(loop_r=None, consts_in_loop=False):
    import concourse.bass as bass
    import concourse.bacc as bacc
    import concourse.mybir as mybir
    import concourse.tile as tile

    f32 = mybir.dt.float32
    mmdt = mybir.dt.bfloat16 if BF16_MM else f32
    nc = bacc.Bacc()

    x_d = nc.dram_tensor("x", [N_PER, 1, 512, 8, 8], f32, kind="ExternalInput")
    w_d = nc.dram_tensor("wall", [128, 64 * NF], mmdt, kind="ExternalInput")
    bias_d = nc.dram_tensor("bias", [1, OUTF], f32, kind="ExternalInput")
    ident_d = nc.dram_tensor("ident", [128, 128], mmdt, kind="ExternalInput")
    ones_d = nc.dram_tensor("ones", [1, 128], f32, kind="ExternalInput")
    out_d = nc.dram_tensor("out", [N_PER, OUTF], f32, kind="ExternalOutput")

    assert sum(CHUNK_SCHED) == 512 and all(c % 8 == 0 for c in CHUNK_SCHED)

    # HBM view: (n, t, h*w); per-(n, chunk) runs are tc*256B contiguous
    x_flat = x_d.rearrange("n one t h w -> n t (one h w)")

    import contextlib
    sizes = sorted(set(CHUNK_SCHED))
    big = max(sizes)
    with tile.TileContext(nc) as tc, contextlib.ExitStack() as ctx:
        consts = ctx.enter_context(tc.tile_pool(name="consts", bufs=1))
        # big chunks: dedicated pool per stream (keeps the two DMA queues
        # decoupled); tail sizes: shared pools (used once per stream)
        rawps = []
        if SHARE_BIG:
            shared_big = ctx.enter_context(
                tc.tile_pool(name=f"raw_{big}", bufs=RAW_BUFS_BIG))
            for nb in range(NB):
                rawps.append({big: shared_big})
        else:
            for nb in range(NB):
                rawps.append({big: ctx.enter_context(
                    tc.tile_pool(name=f"raw{nb}_{big}", bufs=RAW_BUFS_BIG))})
        shared = {
            s: ctx.enter_context(
                tc.tile_pool(name=f"raws{s}", bufs=RAW_BUFS_SMALL))
            for s in sizes if s != big
        }
        for nb in range(NB):
            rawps[nb].update(shared)
        o1p = ctx.enter_context(tc.tile_pool(name="o1", bufs=3))
        pooledp = ctx.enter_context(tc.tile_pool(name="pooled", bufs=3))
        tsbp = ctx.enter_context(tc.tile_pool(name="tsb", bufs=6))
        outp = ctx.enter_context(tc.tile_pool(name="outp", bufs=2))
        tpp = ctx.enter_context(tc.tile_pool(name="tp", bufs=4,
                                             space=bass.MemorySpace.PSUM))
        accp = ctx.enter_context(tc.tile_pool(name="acc", bufs=2,
                                              space=bass.MemorySpace.PSUM))

        w_t = consts.tile([128, 64 * NF], mmdt)
        bias_t = consts.tile([1, OUTF], f32)
        ident_t = consts.tile([128, 128], mmdt)
        ones_t = consts.tile([1, 128], f32)

        def emit_consts():
            # small tensors first: ones/bias feed the first bias matmul and
            # ident the first transpose, all needed ~8us in; w_t (1.5MB)
            # isn't read until the first data matmul right after.
            ceng = getattr(nc, CONST_ENG)
            ceng.dma_start(ones_t[:], ones_d[:])
            ceng.dma_start(bias_t[:], bias_d[:])
            ceng.dma_start(ident_t[:], ident_d[:])
            ceng.dma_start(w_t[:], w_d[:])

        if not consts_in_loop:
            emit_consts()

        loop_cm = tc.For_i(0, loop_r, 1) if loop_r else contextlib.nullcontext()
        with loop_cm:
            if consts_in_loop:
                emit_consts()
            _kernel_body(nc, tc, mybir, f32, mmdt, x_flat, w_t, bias_t,
                         ident_t, ones_t, out_d, rawps, o1p, pooledp, tsbp,
                         outp, tpp, accp)

    nc.compile()
    return nc


def _emit_chunk_compute(nc, f32, mmdt, raw, acc, w_t, ident_t, bias_t, ones_t,
                        t0, tcl, o1p, pooledp, tsbp, tpp, nb,
                        out_d=None, outp=None):
    o1dt = mmdt if O1_BF16 else f32
    pu = min(POOL_UNIT, tcl)
    for u in range(tcl // pu):
        # maxpool over w-pairs (adjacent elements). Tiles are allocated at
        # the max pool-unit size and sliced, so every size shares one tag
        # (tags don't share memory; distinct tags would each cost bufs*size).
        o1f = o1p.tile([128, POOL_UNIT * 32], o1dt, tag="o1", name="o1")
        o1 = o1f[:, :pu * 32]
        r2 = raw[:, u * pu * 64:(u + 1) * pu * 64].rearrange(
            "p (m two) -> p m two", two=2)
        nc.vector.tensor_max(o1, r2[:, :, 0], r2[:, :, 1])

        # maxpool over h-pairs: o1 layout (q, h, ww) -> (blk, hp, ww)
        plf = pooledp.tile([128, POOL_UNIT * 16], mmdt, tag="pl", name="pl")
        pooled = plf[:, :pu * 16]
        o3 = o1.rearrange("p (blk hp ww) -> p blk hp ww", hp=2, ww=4)
        getattr(nc, POOL2_ENG).tensor_max(pooled, o3[:, :, 0, :], o3[:, :, 1, :])

        for ct in range(pu // 8):
            cg = (t0 + u * pu) // 8 + ct   # global 8-t block, 0..63
            b = cg // 16
            if cg % 16 == 0:
                # open this bin's accumulation group with the bias row
                nc.tensor.matmul(
                    acc[:, NF * b:NF * (b + 1)], ones_t[:],
                    bias_t[:, NF * b:NF * (b + 1)],
                    start=True, stop=False)
            tp = tpp.tile([128, 128], mmdt)
            nc.tensor.transpose(tp[:], pooled[:, ct * 128:(ct + 1) * 128],
                                ident_t[:])
            ts = tsbp.tile([128, 128], mmdt)
            if COPY_ENG == "scalar":
                nc.scalar.copy(ts[:], tp[:])
            else:
                getattr(nc, COPY_ENG).tensor_copy(ts[:], tp[:])
            nc.tensor.matmul(
                acc[:, NF * b:NF * (b + 1)], ts[:],
                w_t[:, NF * cg:NF * (cg + 1)],
                start=False, stop=(cg % 16 == 15))
            if OUT_PER_BIN and cg % 16 == 15:
                # this bin's accumulation just closed: LeakyReLU + drain its
                # 48-col output slice now, while the input stream still runs
                sl = slice(NF * b, NF * (b + 1))
                sc = outp.tile([128, NF], f32, tag="sc", name="sc")
                nc.vector.tensor_scalar_mul(sc[:], acc[:, sl], NEG)
                ot = outp.tile([128, NF], f32, tag="ot", name="ot")
                nc.vector.tensor_max(ot[:], acc[:, sl], sc[:])
                oeng = getattr(nc, OUT_ENGS[nb])
                oeng.dma_start(out_d[nb * 128:(nb + 1) * 128, sl], ot[:])


def _emit_tail(nc, f32, acc, out_d, outp, nb):
    # LeakyReLU(z) = max(0.02*z, z) for slope < 1. Both ops on DVE:
    # same-engine in-order execution avoids a cross-engine sem hop in
    # the end-of-kernel critical tail.
    sc = outp.tile([128, OUTF], f32, tag="sc")
    nc.vector.tensor_scalar_mul(sc[:], acc[:], NEG)   # PSUM -> SBUF, *0.02
    ot = outp.tile([128, OUTF], f32, tag="ot")
    nc.vector.tensor_max(ot[:], acc[:], sc[:])
    oeng = getattr(nc, OUT_ENGS[nb])
    oeng.dma_start(out_d[nb * 128:(nb + 1) * 128, :], ot[:])


def _kernel_body(nc, tc, mybir, f32, mmdt, x_flat, w_t, bias_t, ident_t,
                 ones_t, out_d, rawps, o1p, pooledp, tsbp, outp, tpp, accp):
    def dma_in(nb, tcl, t0):
        # shared tag across streams: tags are the allocation unit, so a
        # per-stream tag would double the pool footprint
        raw = rawps[nb][tcl].tile([128, tcl * 64], f32, tag="raw", name="raw")
        eng = getattr(nc, STREAM_ENGS[nb])
        src = x_flat[nb * 128:(nb + 1) * 128, t0:t0 + tcl, :]
        eng.dma_start(raw[:], src.rearrange("p t e -> p (t e)"))
        return raw

    if DMA_ONLY:
        t0 = 0
        for tcl in CHUNK_SCHED:
            for nb in range(NB):
                dma_in(nb, tcl, t0)
            t0 += tcl
        for nb in range(NB):
            oeng = getattr(nc, OUT_ENGS[nb])
            oeng.dma_start(out_d[nb * 128:(nb + 1) * 128, :], w_t[:, :OUTF])
        return

    if INTERLEAVE:
        accs = [accp.tile([128, OUTF], f32, tag="acc", name="acc")
                for nb in range(NB)]
        t0 = 0
        for tcl in CHUNK_SCHED:
            raws = [dma_in(nb, tcl, t0) for nb in range(NB)]
            for nb in range(NB):
                _emit_chunk_compute(nc, f32, mmdt, raws[nb], accs[nb], w_t,
                                    ident_t, bias_t, ones_t, t0, tcl, o1p,
                                    pooledp, tsbp, tpp, nb, out_d, outp)
            t0 += tcl
        if not OUT_PER_BIN:
            for nb in range(NB):
                _emit_tail(nc, f32, accs[nb], out_d, outp, nb)
    else:
        for nb in range(NB):
            acc = accp.tile([128, OUTF], f32)
            t0 = 0
            for tcl in CHUNK_SCHED:
                raw = dma_in(nb, tcl, t0)
                _emit_chunk_compute(nc, f32, mmdt, raw, acc, w_t, ident_t,
                                    bias_t, ones_t, t0, tcl, o1p, pooledp,
                                    tsbp, tpp, nb, out_d, outp)
                t0 += tcl
            if not OUT_PER_BIN:
                _emit_tail(nc, f32, acc, out_d, outp, nb)


def _import_concourse():
    try:
        import concourse.bass_utils  # noqa: F401
    except ImportError:
        import sys
        for p in ("/opt/trn_rl_repo", "/root/.axon_site/_ro/trn_rl_repo"):
            if p not in sys.path:
                sys.path.insert(0, p)
        import concourse.bass_utils  # noqa: F401


def make_in_maps(x, conv_w, conv_b):
    x = np.ascontiguousarray(np.asarray(x), np.float32)
    wall, bias = _prepare_weights(np.asarray(conv_w), np.asarray(conv_b))
    if BF16_MM:
        import ml_dtypes
        wall = wall.astype(ml_dtypes.bfloat16)
        ident = np.eye(128, dtype=ml_dtypes.bfloat16)
    else:
        ident = np.eye(128, dtype=np.float32)
    ones = np.ones((1, 128), np.float32)
    return [
        {"x": np.ascontiguousarray(x[i * N_PER:(i + 1) * N_PER]),
         "wall": wall, "bias": bias, "ident": ident, "ones": ones}
        for i in range(N_CORES)
    ]


def kernel(x, conv_w, conv_b):
    _import_concourse()
    from concourse.bass_utils import run_bass_kernel_spmd

    if "nc" not in _NC_CACHE:
        _NC_CACHE["nc"] = _build_bass()
    nc = _NC_CACHE["nc"]

    in_maps = make_in_maps(x, conv_w, conv_b)
    res = run_bass_kernel_spmd(nc, in_maps, list(range(N_CORES)))
    return np.concatenate([res.results[i]["out"] for i in range(N_CORES)], axis=0)
